# revision 5
# baseline (speedup 1.0000x reference)
"""Trainium2 Bass kernel for nn_DifferentiableSampler — DMA-gather version.

Reference computation (per batch b, sample j):
    locs = clip(point[b,j] + offset[b,j], 0, L-1)
    idx0 = floor(locs); idx1 = idx0 + 1; w1 = locs - idx0; w0 = 1 - w1
    out[b, j, :] = w0 * input[b, :, idx0] + w1 * input[b, :, idx1]

Strategy (pure data parallel over batch, 4 batches per NeuronCore):
  - the host stages each core's input shard TRANSPOSED to [b, L, C] and cast
    to bf16, so one sampling tap = one contiguous 512 B row in DRAM.
  - per batch, ONE gpsimd.dma_gather pulls exactly the 4096 needed rows
    (idx0 rows then idx1 rows) straight into sample-major SBUF layout
    [128, 32, 256]: sample j's g0 row lands at [j % 128, j // 128, :] and
    its g1 row at [j % 128, 16 + j // 128, :].  This reads 2 MiB instead of
    the 4 MiB full image, needs no on-chip transpose, no PSUM, and leaves
    the Vector/Tensor engines almost free.
  - row indices: wrap-layout floor chain -> PE replication matmul -> i16;
    the very first gather granule gets a shortcut chain fed by a tiny
    strided wrap-layout load so the DMA stream starts ~8.7 us in.
  - per 128-sample block q: ACT computes wb = w1 * g1 (per-partition scalar
    weights in the gather's chunk layout), DVE's scalar_tensor_tensor
    finishes out = w0 * g0 + wb straight into the bf16 store tile.
  - output rows are full 256-channel bf16 rows (512 B) stored contiguously;
    the host upcasts bf16 -> f32 while unsharding.

Modeled HBM traffic per core: 8 MiB gathered in + 4 MiB out.
"""

import sys

import numpy as np

if "/opt/trn_rl_repo" not in sys.path:
    sys.path.insert(0, "/opt/trn_rl_repo")

from contextlib import ExitStack

import ml_dtypes
import concourse.bacc as bacc
import concourse.tile as tile
from concourse import masks, mybir
from concourse.bass_utils import run_bass_kernel_spmd

AO = mybir.AluOpType
AF = mybir.ActivationFunctionType
F32 = mybir.dt.float32
BF16 = mybir.dt.bfloat16
I16 = mybir.dt.int16
I32 = mybir.dt.int32

N_CORES = 8
B, C, L, N = 32, 256, 8192, 2048
GAMMA = 1.0  # offset scaling factor
P = 128


def _floor_ops(nc, pool, locs, shape, tag, out=None):
    """Rounding-mode-agnostic floor of a non-negative f32 tile."""
    ri = pool.tile(shape, I32, tag=f"{tag}_ri")
    nc.vector.tensor_copy(ri[:], locs[:])
    rf = pool.tile(shape, F32, tag=f"{tag}_rf")
    nc.vector.tensor_copy(rf[:], ri[:])
    m = pool.tile(shape, F32, tag=f"{tag}_m")
    nc.vector.tensor_tensor(m[:], rf[:], locs[:], op=AO.is_gt)  # 1.0 if r > x
    i0f = out if out is not None else pool.tile(shape, F32, tag=f"{tag}_i0f")
    nc.vector.tensor_tensor(i0f[:], rf[:], m[:], op=AO.subtract)
    return i0f


def _sampler_body(tc, inp, point, offset, out, bpc, c, l, n):
    nc = tc.nc
    n_blk = n // P         # 128-sample blocks (16)
    n_slots = n // 16      # wrap-layout free slots (128)

    with ExitStack() as ctx:
        const_pool = ctx.enter_context(tc.tile_pool(name="const", bufs=1))
        g_pool = ctx.enter_context(tc.tile_pool(name="g", bufs=16))
        meta_pool = ctx.enter_context(tc.tile_pool(name="meta", bufs=4))
        scr_pool = ctx.enter_context(tc.tile_pool(name="scr", bufs=4))
        mps_pool = ctx.enter_context(tc.tile_pool(name="mps", bufs=4,
                                                  space="PSUM"))
        wb_pool = ctx.enter_context(tc.tile_pool(name="wb", bufs=4))
        out_pool = ctx.enter_context(tc.tile_pool(name="outp", bufs=4))

        ident = const_pool.tile([P, P], F32)
        masks.make_identity(nc, ident[:])
        ident16 = const_pool.tile([16, 16], F32)
        masks.make_identity(nc, ident16[:])
        # replication matrix R[16, 128]: R[k, p] = 1 if p % 16 == k
        repl = const_pool.tile([16, P], F32)
        nc.vector.memset(repl[:], 0.0)
        for grp in range(8):
            masks.make_identity(nc, repl[0:16, grp * 16:(grp + 1) * 16],
                                nomemset=True)

        NQ = 4                # gathers per batch (<=1024 idxs: SWDGE ring)
        hb = n_blk // NQ      # blocks per gather granule (4)
        hs = n_slots // NQ    # wrap slots per granule (32)

        def emit_fast_q0():
            """Shortcut index chain for batch 0 quarter 0: a tiny strided
            wrap-layout load of its point+offset feeds the first gather
            without waiting for the full meta load + PE transpose."""
            pq = scr_pool.tile([16, 2, hs], F32, tag="pq0")
            for t in range(2):
                nc.sync.dma_start(
                    pq[:, t, :], point[t, 0, :].rearrange(
                        "(s q) -> q s", q=16)[:, 0:hs])
            sw = scr_pool.tile([16, hs], F32, tag="sw0")
            nc.vector.tensor_tensor(sw[:], pq[:, 0, :], pq[:, 1, :],
                                    op=AO.add)
            lw = scr_pool.tile([16, hs], F32, tag="lw0")
            nc.vector.tensor_scalar(lw[:], sw[:], 0.0,
                                    float(l - 2) + 0.999,
                                    op0=AO.max, op1=AO.min)
            cv0 = scr_pool.tile([16, 2, hs], F32, tag="cv0")
            _floor_ops(nc, scr_pool, lw, [16, hs], "q0", out=cv0[:, 0, :])
            nc.vector.tensor_scalar(cv0[:, 1, :], cv0[:, 0, :], 1.0, None,
                                    op0=AO.add)
            mt0 = mps_pool.tile([P, 2 * hs], F32, tag="mt0")
            nc.tensor.matmul(mt0[:], repl[:], cv0[:, :, :])
            idx16 = meta_pool.tile([P, 2 * hs], I16, tag="idx16q0")
            nc.vector.tensor_copy(idx16[:], mt0[:])
            return idx16

        def emit_meta_dmas():
            """Point+offset (host-stacked in one tensor) in 2 DMAs."""
            # natural-128 layout: partition p holds samples 16p..16p+15
            mA = scr_pool.tile([P, 2, bpc, 16], F32, tag="mA")
            nc.sync.dma_start(mA[:],
                              point.rearrange("t b (p k) -> p t b k", p=P))
            # natural-16 layout: partition k holds samples 128k..128k+127
            mB = scr_pool.tile([16, 2, bpc, P], F32, tag="mB")
            nc.sync.dma_start(mB[:],
                              point.rearrange("t b (k p) -> k t b p", k=16))
            return mA, mB

        def emit_meta(b, state):
            mA, mB = state
            CLAMP = float(l - 2) + 0.999
            sA = scr_pool.tile([P, n // P], F32, tag="sA")
            nc.vector.tensor_tensor(sA[:], mA[:, 0, b, :], mA[:, 1, b, :],
                                    op=AO.add)
            # one PSUM bank holds this batch's meta scratch: psW =
            # mt[0:16, 0:128], psC = mt[:, 128:144], ps_i = mt[:, 144:400]
            mt = mps_pool.tile([P, 512], F32, tag="mt")
            psW = mt[0:16, 0:n_slots]
            nc.tensor.transpose(psW, sA[:], ident[:])
            # indices in wrap layout: locs clamped to [0, l-2+.999] so
            # idx0 = locs - (locs mod 1) is pre-clamped; exact since both
            # operands are exactly representable
            locs_w = scr_pool.tile([16, n_slots], F32, tag="locsw")
            nc.vector.tensor_scalar(locs_w[:], psW, 0.0, CLAMP,
                                    op0=AO.max, op1=AO.min)
            # cvs[:, 0, :] = idx0 slots, cvs[:, 1, :] = idx1 slots; each
            # gather's [idx0-block ; idx1-block] list is a strided sub-view
            cvs = scr_pool.tile([16, 2, n_slots], F32, tag="cvs")
            _floor_ops(nc, scr_pool, locs_w, [16, n_slots], "w",
                       out=cvs[:, 0, :])
            nc.vector.tensor_scalar(cvs[:, 1, :], cvs[:, 0, :], 1.0, None,
                                    op0=AO.add)
            sB = scr_pool.tile([16, n // 16], F32, tag="sB")
            nc.vector.tensor_tensor(sB[:], mB[:, 0, b, :], mB[:, 1, b, :],
                                    op=AO.add)
            psC = mt[:, n_slots:n_slots + n_blk]
            nc.tensor.transpose(psC, sB[:], ident16[:])
            idx16s = []
            for h in range(NQ):
                if b == 0 and h == 0:
                    idx16s.append(emit_meta.q0_idx16)
                    continue
                ps_i = mt[:, 144 + 2 * h * hs:144 + 2 * (h + 1) * hs]
                nc.tensor.matmul(ps_i, repl[:],
                                 cvs[:, :, h * hs:(h + 1) * hs])
                idx16 = meta_pool.tile([P, 2 * hs], I16, tag=f"idx16{h}")
                nc.vector.tensor_copy(idx16[:], ps_i)
                idx16s.append(idx16)
            # weights in chunk layout: w1 = locs - floor(locs)
            locs_j = scr_pool.tile([P, n_blk], F32, tag="locsj")
            nc.vector.tensor_scalar(locs_j[:], psC, 0.0, CLAMP,
                                    op0=AO.max, op1=AO.min)
            i0fj = _floor_ops(nc, scr_pool, locs_j, [P, n_blk], "j")
            w1j = meta_pool.tile([P, n_blk], F32, tag="w1j")
            nc.vector.tensor_tensor(w1j[:], locs_j[:], i0fj[:],
                                    op=AO.subtract)
            w0j = meta_pool.tile([P, n_blk], F32, tag="w0j")
            nc.vector.tensor_scalar(w0j[:], w1j[:], -1.0, 1.0,
                                    op0=AO.mult, op1=AO.add)
            return idx16s, w0j, w1j

        def emit_gather(b, idx16):
            nq = 2 * hb * P  # idxs per granule (1024)
            g = g_pool.tile([P, 2 * hb, c], BF16, tag="g")
            nc.gpsimd.dma_gather(
                g[:], inp[b], idx16[:],
                num_idxs=nq, num_idxs_reg=nq, elem_size=c)
            return g

        def emit_blocks(g, h, mstate, out_big):
            _, w0j, w1j = mstate
            for k in range(h * hb, (h + 1) * hb):
                wb = wb_pool.tile([P, c], BF16, tag="wb")
                nc.scalar.activation(wb[:], g[:, hb + k - h * hb, :], AF.Copy,
                                     scale=w1j[:, k:k + 1])
                nc.vector.scalar_tensor_tensor(
                    out_big[:, k, :], g[:, k - h * hb, :], w0j[:, k:k + 1],
                    wb[:], op0=AO.mult, op1=AO.add)

        # all meta is input-independent: compute the whole index/weight
        # pipeline for every batch up front, then stream gathers/interp
        # in quarter-batch granules so the store tail drains early
        idx16_q0 = emit_fast_q0()
        emit_meta.q0_idx16 = idx16_q0
        g00 = emit_gather(0, idx16_q0)
        state = emit_meta_dmas()
        mstates = [emit_meta(b, state) for b in range(bpc)]
        g_tiles = [[g00 if (b == 0 and h == 0)
                    else emit_gather(b, mstates[b][0][h])
                    for h in range(NQ)] for b in range(bpc)]
        for b in range(bpc):
            out_big = out_pool.tile([P, n_blk, c], BF16, tag="outb")
            out_ap = out[b].rearrange("(k p) cc -> p k cc", p=P)
            for h in range(NQ):
                emit_blocks(g_tiles[b][h], h, mstates[b], out_big)
                nc.sync.dma_start(out_ap[:, h * hb:(h + 1) * hb, :],
                                  out_big[:, h * hb:(h + 1) * hb, :])


def build_nc(bpc, c, l, n):
    nc = bacc.Bacc("TRN2", target_bir_lowering=False, debug=False,
                   num_devices=N_CORES)
    inp = nc.dram_tensor("input", [bpc, l, c], BF16, kind="ExternalInput").ap()
    point = nc.dram_tensor("meta", [2, bpc, n], F32,
                           kind="ExternalInput").ap()
    offset = None
    out = nc.dram_tensor("out", [bpc, n, c], BF16, kind="ExternalOutput").ap()
    with tile.TileContext(nc) as tc:
        _sampler_body(tc, inp, point, offset, out, bpc, c, l, n)
    nc.compile()
    return nc


_NC_CACHE = {}


def _get_nc(bpc=B // N_CORES, c=C, l=L, n=N):
    key = (bpc, c, l, n)
    if key not in _NC_CACHE:
        _NC_CACHE[key] = build_nc(*key)
    return _NC_CACHE[key]


def shard_inputs(input, point, offset):
    input = np.ascontiguousarray(input, dtype=np.float32)
    point = np.ascontiguousarray(point, dtype=np.float32).reshape(B, N)
    offset = np.ascontiguousarray(offset, dtype=np.float32).reshape(B, N)
    bpc = B // N_CORES
    # stage input transposed [b, L, C] as bf16 so each tap is a 512B row
    input_t = np.ascontiguousarray(
        input.transpose(0, 2, 1)).astype(ml_dtypes.bfloat16)
    meta = np.stack([point, offset])  # [2, B, N]
    return [
        {
            "input": input_t[i * bpc:(i + 1) * bpc],
            "meta": meta[:, i * bpc:(i + 1) * bpc],
        }
        for i in range(N_CORES)
    ]


def run_sharded(input, point, offset, trace=False, **kwargs):
    """Run the SPMD kernel on the full inputs; returns (output, results)."""
    nc = _get_nc()
    in_maps = shard_inputs(input, point, offset)
    res = run_bass_kernel_spmd(nc, in_maps, core_ids=list(range(N_CORES)),
                               trace=trace, **kwargs)
    outs = [np.asarray(res.results[i]["out"]).astype(np.float32)
            for i in range(N_CORES)]
    return np.concatenate(outs, axis=0), res


def kernel(input, point, offset):
    out, _ = run_sharded(input, point, offset, trace=False)
    return out


# revision 6
# speedup vs baseline: 1.0322x; 1.0322x over previous
"""Trainium2 Bass kernel for nn_DifferentiableSampler — DMA-gather version.

Reference computation (per batch b, sample j):
    locs = clip(point[b,j] + offset[b,j], 0, L-1)
    idx0 = floor(locs); idx1 = idx0 + 1; w1 = locs - idx0; w0 = 1 - w1
    out[b, j, :] = w0 * input[b, :, idx0] + w1 * input[b, :, idx1]

Strategy (pure data parallel over batch, 4 batches per NeuronCore):
  - the host stages each core's input shard TRANSPOSED to [b, L, C] and cast
    to bf16, so one sampling tap = one contiguous 512 B row in DRAM.
  - per batch, FOUR gpsimd.dma_gathers (1024 indices each — the SWDGE
    descriptor ring holds 1024 descriptors; larger single gathers crash the
    device) pull exactly the 4096 needed rows (idx0 rows then idx1 rows per
    quarter) straight into sample-major SBUF layout [128, 8, 256]: within a
    quarter, sample j's g0 row lands at [j % 128, j // 128, :] and its g1
    row at [j % 128, 4 + j // 128, :].  This reads 2 MiB instead of the
    4 MiB full image, needs no on-chip data transpose, and leaves the
    Vector/Tensor engines mostly free.
  - row indices: wrap-layout floor chain -> PE replication matmul -> i16;
    the very first gather granule gets a shortcut chain fed by a tiny
    strided wrap-layout load so the DMA stream starts ~8.7 us in.
  - per 128-sample block q: ACT computes wb = w1 * g1 (per-partition scalar
    weights in the gather's chunk layout), DVE's scalar_tensor_tensor
    finishes out = w0 * g0 + wb straight into the bf16 store tile.
  - output rows are full 256-channel bf16 rows (512 B) stored contiguously;
    the host upcasts bf16 -> f32 while unsharding.

Modeled HBM traffic per core: 8 MiB gathered in + 4 MiB out.
"""

import sys

import numpy as np

if "/opt/trn_rl_repo" not in sys.path:
    sys.path.insert(0, "/opt/trn_rl_repo")

from contextlib import ExitStack

import ml_dtypes
import concourse.bacc as bacc
import concourse.tile as tile
from concourse import masks, mybir
from concourse.bass_utils import run_bass_kernel_spmd

AO = mybir.AluOpType
AF = mybir.ActivationFunctionType
F32 = mybir.dt.float32
BF16 = mybir.dt.bfloat16
I16 = mybir.dt.int16
I32 = mybir.dt.int32

N_CORES = 8
B, C, L, N = 32, 256, 8192, 2048
GAMMA = 1.0  # offset scaling factor
P = 128


def _floor_ops(nc, pool, locs, shape, tag, out=None):
    """Rounding-mode-agnostic floor of a non-negative f32 tile."""
    ri = pool.tile(shape, I32, tag=f"{tag}_ri")
    nc.vector.tensor_copy(ri[:], locs[:])
    rf = pool.tile(shape, F32, tag=f"{tag}_rf")
    nc.vector.tensor_copy(rf[:], ri[:])
    m = pool.tile(shape, F32, tag=f"{tag}_m")
    nc.vector.tensor_tensor(m[:], rf[:], locs[:], op=AO.is_gt)  # 1.0 if r > x
    i0f = out if out is not None else pool.tile(shape, F32, tag=f"{tag}_i0f")
    nc.vector.tensor_tensor(i0f[:], rf[:], m[:], op=AO.subtract)
    return i0f


def _sampler_body(tc, inp, point, offset, out, bpc, c, l, n):
    nc = tc.nc
    n_blk = n // P         # 128-sample blocks (16)
    n_slots = n // 16      # wrap-layout free slots (128)

    with ExitStack() as ctx:
        const_pool = ctx.enter_context(tc.tile_pool(name="const", bufs=1))
        g_pool = ctx.enter_context(tc.tile_pool(name="g", bufs=16))
        meta_pool = ctx.enter_context(tc.tile_pool(name="meta", bufs=4))
        scr_pool = ctx.enter_context(tc.tile_pool(name="scr", bufs=4))
        mps_pool = ctx.enter_context(tc.tile_pool(name="mps", bufs=4,
                                                  space="PSUM"))
        wb_pool = ctx.enter_context(tc.tile_pool(name="wb", bufs=4))
        out_pool = ctx.enter_context(tc.tile_pool(name="outp", bufs=4))

        ident = const_pool.tile([P, P], F32)
        masks.make_identity(nc, ident[:])
        ident16 = const_pool.tile([16, 16], F32)
        masks.make_identity(nc, ident16[:])
        # replication matrix R[16, 128]: R[k, p] = 1 if p % 16 == k
        repl = const_pool.tile([16, P], F32)
        nc.vector.memset(repl[:], 0.0)
        for grp in range(8):
            masks.make_identity(nc, repl[0:16, grp * 16:(grp + 1) * 16],
                                nomemset=True)

        NQ = 4                # gathers per batch (<=1024 idxs: SWDGE ring)
        hb = n_blk // NQ      # blocks per gather granule (4)
        hs = n_slots // NQ    # wrap slots per granule (32)

        def emit_fast_q0():
            """Shortcut index chain for batch 0 quarter 0: a tiny strided
            wrap-layout load of its point+offset feeds the first gather
            without waiting for the full meta load + PE transpose."""
            pq = scr_pool.tile([16, 2, hs], F32, tag="pq0")
            for t in range(2):
                nc.sync.dma_start(
                    pq[:, t, :], point[t, 0, :].rearrange(
                        "(s q) -> q s", q=16)[:, 0:hs])
            sw = scr_pool.tile([16, hs], F32, tag="sw0")
            nc.vector.tensor_tensor(sw[:], pq[:, 0, :], pq[:, 1, :],
                                    op=AO.add)
            lw = scr_pool.tile([16, hs], F32, tag="lw0")
            nc.vector.tensor_scalar(lw[:], sw[:], 0.0,
                                    float(l - 2) + 0.999,
                                    op0=AO.max, op1=AO.min)
            cv0 = scr_pool.tile([16, 2, hs], F32, tag="cv0")
            _floor_ops(nc, scr_pool, lw, [16, hs], "q0", out=cv0[:, 0, :])
            nc.vector.tensor_scalar(cv0[:, 1, :], cv0[:, 0, :], 1.0, None,
                                    op0=AO.add)
            mt0 = mps_pool.tile([P, 2 * hs], F32, tag="mt0")
            nc.tensor.matmul(mt0[:], repl[:], cv0[:, :, :])
            idx16 = meta_pool.tile([P, 2 * hs], I16, tag="idx16q0")
            nc.vector.tensor_copy(idx16[:], mt0[:])
            return idx16

        def emit_meta_dmas():
            """Point+offset (host-stacked in one tensor) in 2 DMAs."""
            # natural-128 layout: partition p holds samples 16p..16p+15
            mA = scr_pool.tile([P, 2, bpc, 16], F32, tag="mA")
            nc.sync.dma_start(mA[:],
                              point.rearrange("t b (p k) -> p t b k", p=P))
            # natural-16 layout: partition k holds samples 128k..128k+127
            mB = scr_pool.tile([16, 2, bpc, P], F32, tag="mB")
            nc.sync.dma_start(mB[:],
                              point.rearrange("t b (k p) -> k t b p", k=16))
            return mA, mB

        def emit_meta(b, state):
            mA, mB = state
            CLAMP = float(l - 2) + 0.999
            sA = scr_pool.tile([P, n // P], F32, tag="sA")
            nc.vector.tensor_tensor(sA[:], mA[:, 0, b, :], mA[:, 1, b, :],
                                    op=AO.add)
            # one PSUM bank holds this batch's meta scratch: psW =
            # mt[0:16, 0:128], psC = mt[:, 128:144], ps_i = mt[:, 144:400]
            mt = mps_pool.tile([P, 512], F32, tag="mt")
            psW = mt[0:16, 0:n_slots]
            nc.tensor.transpose(psW, sA[:], ident[:])
            # indices in wrap layout: locs clamped to [0, l-2+.999] so
            # idx0 = locs - (locs mod 1) is pre-clamped; exact since both
            # operands are exactly representable
            locs_w = scr_pool.tile([16, n_slots], F32, tag="locsw")
            nc.vector.tensor_scalar(locs_w[:], psW, 0.0, CLAMP,
                                    op0=AO.max, op1=AO.min)
            # cvs[:, 0, :] = idx0 slots, cvs[:, 1, :] = idx1 slots; each
            # gather's [idx0-block ; idx1-block] list is a strided sub-view
            cvs = scr_pool.tile([16, 2, n_slots], F32, tag="cvs")
            _floor_ops(nc, scr_pool, locs_w, [16, n_slots], "w",
                       out=cvs[:, 0, :])
            nc.vector.tensor_scalar(cvs[:, 1, :], cvs[:, 0, :], 1.0, None,
                                    op0=AO.add)
            sB = scr_pool.tile([16, n // 16], F32, tag="sB")
            nc.vector.tensor_tensor(sB[:], mB[:, 0, b, :], mB[:, 1, b, :],
                                    op=AO.add)
            psC = mt[:, n_slots:n_slots + n_blk]
            nc.tensor.transpose(psC, sB[:], ident16[:])
            idx16s = []
            for h in range(NQ):
                if b == 0 and h == 0:
                    idx16s.append(emit_meta.q0_idx16)
                    continue
                ps_i = mt[:, 144 + 2 * h * hs:144 + 2 * (h + 1) * hs]
                nc.tensor.matmul(ps_i, repl[:],
                                 cvs[:, :, h * hs:(h + 1) * hs])
                idx16 = meta_pool.tile([P, 2 * hs], I16, tag=f"idx16{h}")
                nc.vector.tensor_copy(idx16[:], ps_i)
                idx16s.append(idx16)
            # weights in chunk layout: w1 = locs - floor(locs)
            locs_j = scr_pool.tile([P, n_blk], F32, tag="locsj")
            nc.vector.tensor_scalar(locs_j[:], psC, 0.0, CLAMP,
                                    op0=AO.max, op1=AO.min)
            i0fj = _floor_ops(nc, scr_pool, locs_j, [P, n_blk], "j")
            w1j = meta_pool.tile([P, n_blk], F32, tag="w1j")
            nc.vector.tensor_tensor(w1j[:], locs_j[:], i0fj[:],
                                    op=AO.subtract)
            w0j = meta_pool.tile([P, n_blk], F32, tag="w0j")
            nc.vector.tensor_scalar(w0j[:], w1j[:], -1.0, 1.0,
                                    op0=AO.mult, op1=AO.add)
            return idx16s, w0j, w1j

        def emit_gather(b, idx16):
            nq = 2 * hb * P  # idxs per granule (1024)
            g = g_pool.tile([P, 2 * hb, c], BF16, tag="g")
            nc.gpsimd.dma_gather(
                g[:], inp[b], idx16[:],
                num_idxs=nq, num_idxs_reg=nq, elem_size=c)
            return g

        def emit_blocks(g, h, mstate, out_big):
            _, w0j, w1j = mstate
            for k in range(h * hb, (h + 1) * hb):
                wb = wb_pool.tile([P, c], BF16, tag="wb")
                nc.scalar.activation(wb[:], g[:, hb + k - h * hb, :], AF.Copy,
                                     scale=w1j[:, k:k + 1])
                nc.vector.scalar_tensor_tensor(
                    out_big[:, k, :], g[:, k - h * hb, :], w0j[:, k:k + 1],
                    wb[:], op0=AO.mult, op1=AO.add)

        # all meta is input-independent: compute the whole index/weight
        # pipeline for every batch up front, then stream gathers/interp
        # in quarter-batch granules so the store tail drains early
        idx16_q0 = emit_fast_q0()
        emit_meta.q0_idx16 = idx16_q0
        g00 = emit_gather(0, idx16_q0)
        state = emit_meta_dmas()
        mstates = [emit_meta(b, state) for b in range(bpc)]
        g_tiles = [[g00 if (b == 0 and h == 0)
                    else emit_gather(b, mstates[b][0][h])
                    for h in range(NQ)] for b in range(bpc)]
        for b in range(bpc):
            out_big = out_pool.tile([P, n_blk, c], BF16, tag="outb")
            out_ap = out[b].rearrange("(k p) cc -> p k cc", p=P)
            for h in range(NQ):
                emit_blocks(g_tiles[b][h], h, mstates[b], out_big)
                nc.sync.dma_start(out_ap[:, h * hb:(h + 1) * hb, :],
                                  out_big[:, h * hb:(h + 1) * hb, :])


def build_nc(bpc, c, l, n):
    nc = bacc.Bacc("TRN2", target_bir_lowering=False, debug=False,
                   num_devices=N_CORES)
    inp = nc.dram_tensor("input", [bpc, l, c], BF16, kind="ExternalInput").ap()
    point = nc.dram_tensor("meta", [2, bpc, n], F32,
                           kind="ExternalInput").ap()
    offset = None
    out = nc.dram_tensor("out", [bpc, n, c], BF16, kind="ExternalOutput").ap()
    with tile.TileContext(nc) as tc:
        _sampler_body(tc, inp, point, offset, out, bpc, c, l, n)
    nc.compile()
    return nc


_NC_CACHE = {}


def _get_nc(bpc=B // N_CORES, c=C, l=L, n=N):
    key = (bpc, c, l, n)
    if key not in _NC_CACHE:
        _NC_CACHE[key] = build_nc(*key)
    return _NC_CACHE[key]


def shard_inputs(input, point, offset):
    input = np.ascontiguousarray(input, dtype=np.float32)
    point = np.ascontiguousarray(point, dtype=np.float32).reshape(B, N)
    offset = np.ascontiguousarray(offset, dtype=np.float32).reshape(B, N)
    bpc = B // N_CORES
    # stage input transposed [b, L, C] as bf16 so each tap is a 512B row
    input_t = np.ascontiguousarray(
        input.transpose(0, 2, 1)).astype(ml_dtypes.bfloat16)
    meta = np.stack([point, offset])  # [2, B, N]
    return [
        {
            "input": input_t[i * bpc:(i + 1) * bpc],
            "meta": meta[:, i * bpc:(i + 1) * bpc],
        }
        for i in range(N_CORES)
    ]


def run_sharded(input, point, offset, trace=False, **kwargs):
    """Run the SPMD kernel on the full inputs; returns (output, results)."""
    nc = _get_nc()
    in_maps = shard_inputs(input, point, offset)
    res = run_bass_kernel_spmd(nc, in_maps, core_ids=list(range(N_CORES)),
                               trace=trace, **kwargs)
    outs = [np.asarray(res.results[i]["out"]).astype(np.float32)
            for i in range(N_CORES)]
    return np.concatenate(outs, axis=0), res


def kernel(input, point, offset):
    out, _ = run_sharded(input, point, offset, trace=False)
    return out


# revision 7
# speedup vs baseline: 1.0372x; 1.0049x over previous
"""Trainium2 Bass kernel for nn_DifferentiableSampler — DMA-gather version.

Reference computation (per batch b, sample j):
    locs = clip(point[b,j] + offset[b,j], 0, L-1)
    idx0 = floor(locs); idx1 = idx0 + 1; w1 = locs - idx0; w0 = 1 - w1
    out[b, j, :] = w0 * input[b, :, idx0] + w1 * input[b, :, idx1]

Strategy (pure data parallel over batch, 4 batches per NeuronCore):
  - the host stages each core's input shard TRANSPOSED to [b, L, C] and cast
    to bf16, so one sampling tap = one contiguous 512 B row in DRAM.
  - per batch, ONE gpsimd.dma_gather pulls exactly the 4096 needed rows
    (idx0 rows then idx1 rows) straight into sample-major SBUF layout
    [128, 32, 256]: sample j's g0 row lands at [j % 128, j // 128, :] and
    its g1 row at [j % 128, 16 + j // 128, :].  This reads 2 MiB instead of
    the 4 MiB full image, needs no on-chip transpose, no PSUM, and leaves
    the Vector/Tensor engines almost free.
  - row indices: wrap-layout floor chain -> PE replication matmul -> i16.
  - per 128-sample block q: ACT computes wb = w1 * g1 (per-partition scalar
    weights in the gather's chunk layout), DVE's scalar_tensor_tensor
    finishes out = w0 * g0 + wb straight into the bf16 store tile.
  - output rows are full 256-channel bf16 rows (512 B) stored contiguously;
    the host upcasts bf16 -> f32 while unsharding.

Modeled HBM traffic per core: 8 MiB gathered in + 4 MiB out.
"""

import sys

import numpy as np

if "/opt/trn_rl_repo" not in sys.path:
    sys.path.insert(0, "/opt/trn_rl_repo")

from contextlib import ExitStack

import ml_dtypes
import concourse.bacc as bacc
import concourse.tile as tile
from concourse import masks, mybir
from concourse.bass_utils import run_bass_kernel_spmd

AO = mybir.AluOpType
AF = mybir.ActivationFunctionType
F32 = mybir.dt.float32
BF16 = mybir.dt.bfloat16
I16 = mybir.dt.int16
I32 = mybir.dt.int32

N_CORES = 8
B, C, L, N = 32, 256, 8192, 2048
GAMMA = 1.0  # offset scaling factor
P = 128


def _floor_ops(nc, pool, locs, shape, tag, out=None):
    """Rounding-mode-agnostic floor of a non-negative f32 tile."""
    ri = pool.tile(shape, I32, tag=f"{tag}_ri")
    nc.vector.tensor_copy(ri[:], locs[:])
    rf = pool.tile(shape, F32, tag=f"{tag}_rf")
    nc.vector.tensor_copy(rf[:], ri[:])
    m = pool.tile(shape, F32, tag=f"{tag}_m")
    nc.vector.tensor_tensor(m[:], rf[:], locs[:], op=AO.is_gt)  # 1.0 if r > x
    i0f = out if out is not None else pool.tile(shape, F32, tag=f"{tag}_i0f")
    nc.vector.tensor_tensor(i0f[:], rf[:], m[:], op=AO.subtract)
    return i0f


def _sampler_body(tc, inp, point, offset, out, bpc, c, l, n):
    nc = tc.nc
    n_blk = n // P         # 128-sample blocks (16)
    n_slots = n // 16      # wrap-layout free slots (128)

    with ExitStack() as ctx:
        const_pool = ctx.enter_context(tc.tile_pool(name="const", bufs=1))
        g_pool = ctx.enter_context(tc.tile_pool(name="g", bufs=16))
        meta_pool = ctx.enter_context(tc.tile_pool(name="meta", bufs=4))
        scr_pool = ctx.enter_context(tc.tile_pool(name="scr", bufs=4))
        mps_pool = ctx.enter_context(tc.tile_pool(name="mps", bufs=4,
                                                  space="PSUM"))
        wb_pool = ctx.enter_context(tc.tile_pool(name="wb", bufs=4))
        out_pool = ctx.enter_context(tc.tile_pool(name="outp", bufs=4))

        ident = const_pool.tile([P, P], F32)
        masks.make_identity(nc, ident[:])
        ident16 = const_pool.tile([16, 16], F32)
        masks.make_identity(nc, ident16[:])
        # replication matrix R[16, 128]: R[k, p] = 1 if p % 16 == k
        repl = const_pool.tile([16, P], F32)
        nc.vector.memset(repl[:], 0.0)
        ctx0 = const_pool.tile([P, n // P], mybir.dt.int32)
        nc.vector.memset(ctx0[:], 0)
        for grp in range(8):
            masks.make_identity(nc, repl[0:16, grp * 16:(grp + 1) * 16],
                                nomemset=True)

        NQ = 4                # gathers per batch (<=1024 idxs: SWDGE ring)
        N_T2 = 1              # type-2 blocks per gather granule
        hb = n_blk // NQ      # blocks per gather granule (4)
        hs = n_slots // NQ    # wrap slots per granule (32)

        def emit_fast_q0():
            """Shortcut index chain for batch 0 quarter 0: a tiny strided
            wrap-layout load of its point+offset feeds the first gather
            without waiting for the full meta load + PE transpose."""
            pq = scr_pool.tile([16, 2, hs], F32, tag="pq0")
            for t in range(2):
                nc.sync.dma_start(
                    pq[:, t, :], point[t, 0, :].rearrange(
                        "(s q) -> q s", q=16)[:, 0:hs])
            sw = scr_pool.tile([16, hs], F32, tag="sw0")
            nc.vector.tensor_tensor(sw[:], pq[:, 0, :], pq[:, 1, :],
                                    op=AO.add)
            lw = scr_pool.tile([16, hs], F32, tag="lw0")
            nc.vector.tensor_scalar(lw[:], sw[:], 0.0,
                                    float(l - 2) + 0.999,
                                    op0=AO.max, op1=AO.min)
            cv0 = scr_pool.tile([16, 2, hs], F32, tag="cv0")
            _floor_ops(nc, scr_pool, lw, [16, hs], "q0", out=cv0[:, 0, :])
            nc.vector.tensor_scalar(cv0[:, 1, :], cv0[:, 0, :], 1.0, None,
                                    op0=AO.add)
            mt0 = mps_pool.tile([P, 2 * hs], F32, tag="mt0")
            nc.tensor.matmul(mt0[:], repl[:], cv0[:, :, :])
            idx16 = meta_pool.tile([P, 2 * hs], I16, tag="idx16q0")
            nc.vector.tensor_copy(idx16[:], mt0[:])
            return idx16

        def emit_meta_dmas():
            """Point+offset (host-stacked in one tensor) in 2 DMAs."""
            # natural-128 layout: partition p holds samples 16p..16p+15
            mA = scr_pool.tile([P, 2, bpc, 16], F32, tag="mA")
            nc.sync.dma_start(mA[:],
                              point.rearrange("t b (p k) -> p t b k", p=P))
            # natural-16 layout: partition k holds samples 128k..128k+127
            mB = scr_pool.tile([16, 2, bpc, P], F32, tag="mB")
            nc.sync.dma_start(mB[:],
                              point.rearrange("t b (k p) -> k t b p", k=16))
            return mA, mB

        def emit_meta(b, state):
            mA, mB = state
            CLAMP = float(l - 2) + 0.999
            sA = scr_pool.tile([P, n // P], F32, tag="sA")
            nc.vector.tensor_tensor(sA[:], mA[:, 0, b, :], mA[:, 1, b, :],
                                    op=AO.add)
            # one PSUM bank holds this batch's meta scratch: psW =
            # mt[0:16, 0:128], psC = mt[:, 128:144], ps_i = mt[:, 144:400]
            mt = mps_pool.tile([P, 512], F32, tag="mt")
            psW = mt[0:16, 0:n_slots]
            nc.tensor.transpose(psW, sA[:], ident[:])
            # indices in wrap layout: locs clamped to [0, l-2+.999] so
            # idx0 = locs - (locs mod 1) is pre-clamped; exact since both
            # operands are exactly representable
            locs_w = scr_pool.tile([16, n_slots], F32, tag="locsw")
            nc.vector.tensor_scalar(locs_w[:], psW, 0.0, CLAMP,
                                    op0=AO.max, op1=AO.min)
            # cvs[:, 0, :] = idx0 slots, cvs[:, 1, :] = idx1 slots; each
            # gather's [idx0-block ; idx1-block] list is a strided sub-view
            cvs = scr_pool.tile([16, 2, n_slots], F32, tag="cvs")
            _floor_ops(nc, scr_pool, locs_w, [16, n_slots], "w",
                       out=cvs[:, 0, :])
            nc.vector.tensor_scalar(cvs[:, 1, :], cvs[:, 0, :], 1.0, None,
                                    op0=AO.add)
            sB = scr_pool.tile([16, n // 16], F32, tag="sB")
            nc.vector.tensor_tensor(sB[:], mB[:, 0, b, :], mB[:, 1, b, :],
                                    op=AO.add)
            psC = mt[:, n_slots:n_slots + n_blk]
            nc.tensor.transpose(psC, sB[:], ident16[:])
            idx16s = []
            for h in range(NQ):
                if b == 0 and h == 0:
                    idx16s.append(emit_meta.q0_idx16)
                    continue
                ps_i = mt[:, 144 + 2 * h * hs:144 + 2 * (h + 1) * hs]
                nc.tensor.matmul(ps_i, repl[:],
                                 cvs[:, :, h * hs:(h + 1) * hs])
                idx16 = meta_pool.tile([P, 2 * hs], I16, tag=f"idx16{h}")
                nc.vector.tensor_copy(idx16[:], ps_i)
                idx16s.append(idx16)
            # weights in chunk layout: w1 = locs - floor(locs)
            locs_j = scr_pool.tile([P, n_blk], F32, tag="locsj")
            nc.vector.tensor_scalar(locs_j[:], psC, 0.0, CLAMP,
                                    op0=AO.max, op1=AO.min)
            i0fj = _floor_ops(nc, scr_pool, locs_j, [P, n_blk], "j")
            w1j = meta_pool.tile([P, n_blk], F32, tag="w1j")
            nc.vector.tensor_tensor(w1j[:], locs_j[:], i0fj[:],
                                    op=AO.subtract)
            w0j = meta_pool.tile([P, n_blk], F32, tag="w0j")
            nc.vector.tensor_scalar(w0j[:], w1j[:], -1.0, 1.0,
                                    op0=AO.mult, op1=AO.add)
            return idx16s, w0j, w1j

        def emit_gather(b, idx16):
            nq = 2 * hb * P  # idxs per granule (1024)
            g = g_pool.tile([P, 2 * hb, c], BF16, tag="g")
            nc.gpsimd.dma_gather(
                g[:], inp[b], idx16[:],
                num_idxs=nq, num_idxs_reg=nq, elem_size=c)
            return g

        def emit_blocks(g, h, mstate, out_big, n_t2=0):
            _, w0j, w1j = mstate
            for k in range(h * hb, (h + 1) * hb):
                wb = wb_pool.tile([P, c], BF16, tag="wb")
                nc.scalar.activation(wb[:], g[:, hb + k - h * hb, :], AF.Copy,
                                     scale=w1j[:, k:k + 1])
                if k - h * hb < n_t2:
                    # type-2 block: both scales on ACT, bf16 2x-mode add on
                    # DVE — shifts load off the binding DVE stream
                    wa = wb_pool.tile([P, c], BF16, tag="wa")
                    nc.scalar.activation(wa[:], g[:, k - h * hb, :], AF.Copy,
                                         scale=w0j[:, k:k + 1])
                    nc.vector.tensor_tensor(out_big[:, k, :], wa[:], wb[:],
                                            op=AO.add)
                else:
                    nc.vector.scalar_tensor_tensor(
                        out_big[:, k, :], g[:, k - h * hb, :],
                        w0j[:, k:k + 1], wb[:], op0=AO.mult, op1=AO.add)

        # all meta is input-independent: compute the whole index/weight
        # pipeline for every batch up front, then stream gathers/interp
        # in quarter-batch granules so the store tail drains early
        idx16_q0 = emit_fast_q0()
        emit_meta.q0_idx16 = idx16_q0
        g00 = emit_gather(0, idx16_q0)
        state = emit_meta_dmas()
        mstates = [emit_meta(b, state) for b in range(bpc)]
        g_tiles = [[g00 if (b == 0 and h == 0)
                    else emit_gather(b, mstates[b][0][h])
                    for h in range(NQ)] for b in range(bpc)]
        for b in range(bpc):
            out_big = out_pool.tile([P, n_blk, c], BF16, tag="outb")
            for h in range(NQ):
                emit_blocks(g_tiles[b][h], h, mstates[b], out_big,
                            n_t2=N_T2)
            # indexed KV-writeback doubles as a plain grouped row store:
            # "batch" = 16 sample groups, d_head = 128 samples (partitions),
            # n_ctx = the 256 contiguous channels, ctx_idx = 0
            nc.gpsimd.kv_writeback(
                out[b].rearrange("(g p) (one cc) -> g p one cc",
                                 p=P, one=1),
                out_big.rearrange("p (one g) cc -> p one g cc", one=1),
                ctx0[:, 0:n_blk])


def build_nc(bpc, c, l, n):
    nc = bacc.Bacc("TRN2", target_bir_lowering=False, debug=False,
                   num_devices=N_CORES)
    inp = nc.dram_tensor("input", [bpc, l, c], BF16, kind="ExternalInput").ap()
    point = nc.dram_tensor("meta", [2, bpc, n], F32,
                           kind="ExternalInput").ap()
    offset = None
    out = nc.dram_tensor("out", [bpc, n, c], BF16, kind="ExternalOutput").ap()
    with tile.TileContext(nc) as tc:
        _sampler_body(tc, inp, point, offset, out, bpc, c, l, n)
    nc.compile()
    return nc


_NC_CACHE = {}


def _get_nc(bpc=B // N_CORES, c=C, l=L, n=N):
    key = (bpc, c, l, n)
    if key not in _NC_CACHE:
        _NC_CACHE[key] = build_nc(*key)
    return _NC_CACHE[key]


def shard_inputs(input, point, offset):
    input = np.ascontiguousarray(input, dtype=np.float32)
    point = np.ascontiguousarray(point, dtype=np.float32).reshape(B, N)
    offset = np.ascontiguousarray(offset, dtype=np.float32).reshape(B, N)
    bpc = B // N_CORES
    # stage input transposed [b, L, C] as bf16 so each tap is a 512B row
    input_t = np.ascontiguousarray(
        input.transpose(0, 2, 1)).astype(ml_dtypes.bfloat16)
    meta = np.stack([point, offset])  # [2, B, N]
    return [
        {
            "input": input_t[i * bpc:(i + 1) * bpc],
            "meta": meta[:, i * bpc:(i + 1) * bpc],
        }
        for i in range(N_CORES)
    ]


def run_sharded(input, point, offset, trace=False, **kwargs):
    """Run the SPMD kernel on the full inputs; returns (output, results)."""
    nc = _get_nc()
    in_maps = shard_inputs(input, point, offset)
    res = run_bass_kernel_spmd(nc, in_maps, core_ids=list(range(N_CORES)),
                               trace=trace, **kwargs)
    outs = [np.asarray(res.results[i]["out"]).astype(np.float32)
            for i in range(N_CORES)]
    return np.concatenate(outs, axis=0), res


def kernel(input, point, offset):
    out, _ = run_sharded(input, point, offset, trace=False)
    return out


# revision 8
# speedup vs baseline: 1.0658x; 1.0276x over previous
"""Trainium2 Bass kernel for nn_DifferentiableSampler — DMA-gather version.

Reference computation (per batch b, sample j):
    locs = clip(point[b,j] + offset[b,j], 0, L-1)
    idx0 = floor(locs); idx1 = idx0 + 1; w1 = locs - idx0; w0 = 1 - w1
    out[b, j, :] = w0 * input[b, :, idx0] + w1 * input[b, :, idx1]

Strategy (pure data parallel over batch, 4 batches per NeuronCore):
  - the host stages each core's input shard TRANSPOSED to [b, L, C] and cast
    to bf16, so one sampling tap = one contiguous 512 B row in DRAM.
  - per batch, FOUR gpsimd.dma_gathers (1024 indices each — the SWDGE
    descriptor ring holds 1024 descriptors; larger single gathers crash the
    device) pull exactly the 4096 needed rows (idx0 rows then idx1 rows per
    quarter) straight into sample-major SBUF layout [128, 8, 256]: within a
    quarter, sample j's g0 row lands at [j % 128, j // 128, :] and its g1
    row at [j % 128, 4 + j // 128, :].  This reads 2 MiB instead of the
    4 MiB full image and needs no on-chip data transpose.
  - row indices: wrap-layout floor chain -> PE replication matmul -> i16;
    the first gather granule gets a shortcut chain fed by a tiny strided
    wrap-layout load so the DMA stream starts as early as possible.
  - per 128-sample block q: ACT computes wb = w1 * g1 (per-partition scalar
    weights in the gather's chunk layout), DVE's scalar_tensor_tensor
    finishes out = w0 * g0 + wb straight into the bf16 store tile.
  - output rows leave via gpsimd.kv_writeback (ctx_idx = 0, n_ctx = C):
    semantically a plain grouped row store, but descriptor-generated in
    16-lane SIMD so it is far cheaper on the DMA engines than a regular
    dma_start; the host upcasts bf16 -> f32 while unsharding.

Modeled HBM traffic per core: 8 MiB gathered in + 4 MiB out.
"""

import sys

import numpy as np

if "/opt/trn_rl_repo" not in sys.path:
    sys.path.insert(0, "/opt/trn_rl_repo")

from contextlib import ExitStack

import ml_dtypes
import concourse.bacc as bacc
import concourse.tile as tile
from concourse import masks, mybir
from concourse.bass_utils import run_bass_kernel_spmd

AO = mybir.AluOpType
AF = mybir.ActivationFunctionType
F32 = mybir.dt.float32
BF16 = mybir.dt.bfloat16
I16 = mybir.dt.int16
I32 = mybir.dt.int32

N_CORES = 8
B, C, L, N = 32, 256, 8192, 2048
GAMMA = 1.0  # offset scaling factor
P = 128


def _floor_ops(nc, pool, locs, shape, tag, out=None):
    """Rounding-mode-agnostic floor of a non-negative f32 tile."""
    ri = pool.tile(shape, I32, tag=f"{tag}_ri")
    nc.vector.tensor_copy(ri[:], locs[:])
    rf = pool.tile(shape, F32, tag=f"{tag}_rf")
    nc.vector.tensor_copy(rf[:], ri[:])
    m = pool.tile(shape, F32, tag=f"{tag}_m")
    nc.vector.tensor_tensor(m[:], rf[:], locs[:], op=AO.is_gt)  # 1.0 if r > x
    i0f = out if out is not None else pool.tile(shape, F32, tag=f"{tag}_i0f")
    nc.vector.tensor_tensor(i0f[:], rf[:], m[:], op=AO.subtract)
    return i0f


def _sampler_body(tc, inp, point, offset, out, bpc, c, l, n):
    nc = tc.nc
    n_blk = n // P         # 128-sample blocks (16)
    n_slots = n // 16      # wrap-layout free slots (128)

    with ExitStack() as ctx:
        const_pool = ctx.enter_context(tc.tile_pool(name="const", bufs=1))
        g_pool = ctx.enter_context(tc.tile_pool(name="g", bufs=16))
        meta_pool = ctx.enter_context(tc.tile_pool(name="meta", bufs=4))
        scr_pool = ctx.enter_context(tc.tile_pool(name="scr", bufs=4))
        mps_pool = ctx.enter_context(tc.tile_pool(name="mps", bufs=4,
                                                  space="PSUM"))
        wb_pool = ctx.enter_context(tc.tile_pool(name="wb", bufs=4))
        out_pool = ctx.enter_context(tc.tile_pool(name="outp", bufs=4))

        ident = const_pool.tile([P, P], F32)
        masks.make_identity(nc, ident[:])
        ident16 = const_pool.tile([16, 16], F32)
        masks.make_identity(nc, ident16[:])
        # replication matrix R[16, 128]: R[k, p] = 1 if p % 16 == k
        repl = const_pool.tile([16, P], F32)
        nc.vector.memset(repl[:], 0.0)
        ctx0 = const_pool.tile([P, n // P], mybir.dt.int32)
        nc.vector.memset(ctx0[:], 0)
        for grp in range(8):
            masks.make_identity(nc, repl[0:16, grp * 16:(grp + 1) * 16],
                                nomemset=True)

        NQ = 4                # gathers per batch (<=1024 idxs: SWDGE ring)
        N_T2 = 1              # type-2 blocks per gather granule
        hb = n_blk // NQ      # blocks per gather granule (4)
        hs = n_slots // NQ    # wrap slots per granule (32)

        def emit_fast_q0():
            """Shortcut index chain for batch 0 quarter 0: a tiny strided
            wrap-layout load of its point+offset feeds the first gather
            without waiting for the full meta load + PE transpose."""
            pq = scr_pool.tile([16, 2, hs], F32, tag="pq0")
            for t in range(2):
                nc.sync.dma_start(
                    pq[:, t, :], point[t, 0, :].rearrange(
                        "(s q) -> q s", q=16)[:, 0:hs])
            sw = scr_pool.tile([16, hs], F32, tag="sw0")
            nc.vector.tensor_tensor(sw[:], pq[:, 0, :], pq[:, 1, :],
                                    op=AO.add)
            lw = scr_pool.tile([16, hs], F32, tag="lw0")
            nc.vector.tensor_scalar(lw[:], sw[:], 0.0,
                                    float(l - 2) + 0.999,
                                    op0=AO.max, op1=AO.min)
            cv0 = scr_pool.tile([16, 2, hs], F32, tag="cv0")
            _floor_ops(nc, scr_pool, lw, [16, hs], "q0", out=cv0[:, 0, :])
            nc.vector.tensor_scalar(cv0[:, 1, :], cv0[:, 0, :], 1.0, None,
                                    op0=AO.add)
            mt0 = mps_pool.tile([P, 2 * hs], F32, tag="mt0")
            nc.tensor.matmul(mt0[:], repl[:], cv0[:, :, :])
            idx16 = meta_pool.tile([P, 2 * hs], I16, tag="idx16q0")
            nc.vector.tensor_copy(idx16[:], mt0[:])
            return idx16

        def emit_meta_dmas():
            """Point+offset (host-stacked in one tensor) in 2 DMAs."""
            # natural-128 layout: partition p holds samples 16p..16p+15
            mA = scr_pool.tile([P, 2, bpc, 16], F32, tag="mA")
            nc.sync.dma_start(mA[:],
                              point.rearrange("t b (p k) -> p t b k", p=P))
            # natural-16 layout: partition k holds samples 128k..128k+127
            mB = scr_pool.tile([16, 2, bpc, P], F32, tag="mB")
            nc.sync.dma_start(mB[:],
                              point.rearrange("t b (k p) -> k t b p", k=16))
            return mA, mB

        def emit_meta(b, state):
            mA, mB = state
            CLAMP = float(l - 2) + 0.999
            sA = scr_pool.tile([P, n // P], F32, tag="sA")
            nc.vector.tensor_tensor(sA[:], mA[:, 0, b, :], mA[:, 1, b, :],
                                    op=AO.add)
            # one PSUM bank holds this batch's meta scratch: psW =
            # mt[0:16, 0:128], psC = mt[:, 128:144], ps_i = mt[:, 144:400]
            mt = mps_pool.tile([P, 512], F32, tag="mt")
            psW = mt[0:16, 0:n_slots]
            nc.tensor.transpose(psW, sA[:], ident[:])
            # indices in wrap layout: locs clamped to [0, l-2+.999] so
            # idx0 = locs - (locs mod 1) is pre-clamped; exact since both
            # operands are exactly representable
            locs_w = scr_pool.tile([16, n_slots], F32, tag="locsw")
            nc.vector.tensor_scalar(locs_w[:], psW, 0.0, CLAMP,
                                    op0=AO.max, op1=AO.min)
            # cvs[:, 0, :] = idx0 slots, cvs[:, 1, :] = idx1 slots; each
            # gather's [idx0-block ; idx1-block] list is a strided sub-view
            cvs = scr_pool.tile([16, 2, n_slots], F32, tag="cvs")
            _floor_ops(nc, scr_pool, locs_w, [16, n_slots], "w",
                       out=cvs[:, 0, :])
            nc.vector.tensor_scalar(cvs[:, 1, :], cvs[:, 0, :], 1.0, None,
                                    op0=AO.add)
            sB = scr_pool.tile([16, n // 16], F32, tag="sB")
            nc.vector.tensor_tensor(sB[:], mB[:, 0, b, :], mB[:, 1, b, :],
                                    op=AO.add)
            psC = mt[:, n_slots:n_slots + n_blk]
            nc.tensor.transpose(psC, sB[:], ident16[:])
            idx16s = []
            for h in range(NQ):
                if b == 0 and h == 0:
                    idx16s.append(emit_meta.q0_idx16)
                    continue
                ps_i = mt[:, 144 + 2 * h * hs:144 + 2 * (h + 1) * hs]
                nc.tensor.matmul(ps_i, repl[:],
                                 cvs[:, :, h * hs:(h + 1) * hs])
                idx16 = meta_pool.tile([P, 2 * hs], I16, tag=f"idx16{h}")
                nc.vector.tensor_copy(idx16[:], ps_i)
                idx16s.append(idx16)
            # weights in chunk layout: w1 = locs - floor(locs)
            locs_j = scr_pool.tile([P, n_blk], F32, tag="locsj")
            nc.vector.tensor_scalar(locs_j[:], psC, 0.0, CLAMP,
                                    op0=AO.max, op1=AO.min)
            i0fj = _floor_ops(nc, scr_pool, locs_j, [P, n_blk], "j")
            w1j = meta_pool.tile([P, n_blk], F32, tag="w1j")
            nc.vector.tensor_tensor(w1j[:], locs_j[:], i0fj[:],
                                    op=AO.subtract)
            w0j = meta_pool.tile([P, n_blk], F32, tag="w0j")
            nc.vector.tensor_scalar(w0j[:], w1j[:], -1.0, 1.0,
                                    op0=AO.mult, op1=AO.add)
            return idx16s, w0j, w1j

        def emit_gather(b, idx16):
            nq = 2 * hb * P  # idxs per granule (1024)
            g = g_pool.tile([P, 2 * hb, c], BF16, tag="g")
            nc.gpsimd.dma_gather(
                g[:], inp[b], idx16[:],
                num_idxs=nq, num_idxs_reg=nq, elem_size=c)
            return g

        def emit_blocks(g, h, mstate, out_big, n_t2=0):
            _, w0j, w1j = mstate
            for k in range(h * hb, (h + 1) * hb):
                wb = wb_pool.tile([P, c], BF16, tag="wb")
                nc.scalar.activation(wb[:], g[:, hb + k - h * hb, :], AF.Copy,
                                     scale=w1j[:, k:k + 1])
                if k - h * hb < n_t2:
                    # type-2 block: both scales on ACT, bf16 2x-mode add on
                    # DVE — shifts load off the binding DVE stream
                    wa = wb_pool.tile([P, c], BF16, tag="wa")
                    nc.scalar.activation(wa[:], g[:, k - h * hb, :], AF.Copy,
                                         scale=w0j[:, k:k + 1])
                    nc.vector.tensor_tensor(out_big[:, k, :], wa[:], wb[:],
                                            op=AO.add)
                else:
                    nc.vector.scalar_tensor_tensor(
                        out_big[:, k, :], g[:, k - h * hb, :],
                        w0j[:, k:k + 1], wb[:], op0=AO.mult, op1=AO.add)

        # all meta is input-independent: compute the whole index/weight
        # pipeline for every batch up front, then stream gathers/interp
        # in quarter-batch granules so the store tail drains early
        idx16_q0 = emit_fast_q0()
        emit_meta.q0_idx16 = idx16_q0
        g00 = emit_gather(0, idx16_q0)
        state = emit_meta_dmas()
        mstates = [emit_meta(b, state) for b in range(bpc)]
        g_tiles = [[g00 if (b == 0 and h == 0)
                    else emit_gather(b, mstates[b][0][h])
                    for h in range(NQ)] for b in range(bpc)]
        for b in range(bpc):
            out_big = out_pool.tile([P, n_blk, c], BF16, tag="outb")
            out4 = out[b].rearrange("(g p) (one cc) -> g p one cc",
                                    p=P, one=1)
            big4 = out_big.rearrange("p (one g) cc -> p one g cc", one=1)
            hg = n_blk // 2
            for h in range(NQ):
                emit_blocks(g_tiles[b][h], h, mstates[b], out_big,
                            n_t2=N_T2)
                if h % 2 == 1:
                    # indexed KV-writeback doubles as a grouped row store:
                    # "batch" = sample groups, d_head = 128 samples,
                    # n_ctx = the 256 contiguous channels, ctx_idx = 0
                    s = (h // 2) * hg
                    nc.gpsimd.kv_writeback(
                        out4[s:s + hg], big4[:, :, s:s + hg, :],
                        ctx0[:, 0:hg])


def build_nc(bpc, c, l, n):
    nc = bacc.Bacc("TRN2", target_bir_lowering=False, debug=False,
                   num_devices=N_CORES)
    inp = nc.dram_tensor("input", [bpc, l, c], BF16, kind="ExternalInput").ap()
    point = nc.dram_tensor("meta", [2, bpc, n], F32,
                           kind="ExternalInput").ap()
    offset = None
    out = nc.dram_tensor("out", [bpc, n, c], BF16, kind="ExternalOutput").ap()
    with tile.TileContext(nc) as tc:
        _sampler_body(tc, inp, point, offset, out, bpc, c, l, n)
    nc.compile()
    return nc


_NC_CACHE = {}


def _get_nc(bpc=B // N_CORES, c=C, l=L, n=N):
    key = (bpc, c, l, n)
    if key not in _NC_CACHE:
        _NC_CACHE[key] = build_nc(*key)
    return _NC_CACHE[key]


def shard_inputs(input, point, offset):
    input = np.ascontiguousarray(input, dtype=np.float32)
    point = np.ascontiguousarray(point, dtype=np.float32).reshape(B, N)
    offset = np.ascontiguousarray(offset, dtype=np.float32).reshape(B, N)
    bpc = B // N_CORES
    # stage input transposed [b, L, C] as bf16 so each tap is a 512B row
    input_t = np.ascontiguousarray(
        input.transpose(0, 2, 1)).astype(ml_dtypes.bfloat16)
    meta = np.stack([point, offset])  # [2, B, N]
    return [
        {
            "input": input_t[i * bpc:(i + 1) * bpc],
            "meta": meta[:, i * bpc:(i + 1) * bpc],
        }
        for i in range(N_CORES)
    ]


def run_sharded(input, point, offset, trace=False, **kwargs):
    """Run the SPMD kernel on the full inputs; returns (output, results)."""
    nc = _get_nc()
    in_maps = shard_inputs(input, point, offset)
    res = run_bass_kernel_spmd(nc, in_maps, core_ids=list(range(N_CORES)),
                               trace=trace, **kwargs)
    outs = [np.asarray(res.results[i]["out"]).astype(np.float32)
            for i in range(N_CORES)]
    return np.concatenate(outs, axis=0), res


def kernel(input, point, offset):
    out, _ = run_sharded(input, point, offset, trace=False)
    return out
